# revision 3
# baseline (speedup 1.0000x reference)
"""Trainium2 Bass kernel for HarebrainedPad2d.

Five "earth" strips at different resolutions each get circular-W / zero-H
padding (PAD=2); the 2-row top/bottom pad edges are then overwritten with
small stride-2 convs (to_outer) / transposed convs (to_inner) of the
neighboring strips' edge rows.

Strategy: shard the longitude (W) axis across 8 NeuronCores. The host
slices each strip per core with a 2-column circular halo (so no device
collectives are needed), each core produces its exact W/8-column chunk of
all five padded outputs (bulk = DRAM->DRAM body copies; edges = TensorE
tap matmuls), and the host stitches the chunks back together. The padded
output's last 4 columns circularly duplicate its first 4, so the host
fills those by copying.
"""
import numpy as np

C = 256
HS = [30, 60, 180, 60, 30]
WS = [360, 720, 1440, 720, 360]
NCORES = 8
W8 = [w // 8 for w in WS]          # per-core output chunk widths [45,90,180,90,45]
BW = [w + 2 for w in W8]           # body input widths (2-col left halo + body)

_cached_nc = None


def _build():
    import concourse.bacc as bacc
    import concourse.mybir as mybir
    from concourse.tile import TileContext

    f32 = mybir.dt.float32
    nc = bacc.Bacc("TRN2")

    b = [
        nc.dram_tensor(f"b{i}", (C, HS[i], BW[i]), f32, kind="ExternalInput")
        for i in range(5)
    ]
    e5 = nc.dram_tensor("e5", (C, 2, 183), f32, kind="ExternalInput")
    e6 = nc.dram_tensor("e6", (C, 2, 183), f32, kind="ExternalInput")
    e7 = nc.dram_tensor("e7", (C, 2, 93), f32, kind="ExternalInput")
    e8 = nc.dram_tensor("e8", (C, 2, 93), f32, kind="ExternalInput")
    wA0 = nc.dram_tensor("wA0", (128, 2, 5, 2, 128), f32, kind="ExternalInput")
    wA1 = nc.dram_tensor("wA1", (128, 2, 5, 2, 128), f32, kind="ExternalInput")
    wB0 = nc.dram_tensor("wB0", (128, 2, 5, 2, 128), f32, kind="ExternalInput")
    wB1 = nc.dram_tensor("wB1", (128, 2, 5, 2, 128), f32, kind="ExternalInput")
    bias0 = nc.dram_tensor("bias0", (128, 2), f32, kind="ExternalInput")
    bias1 = nc.dram_tensor("bias1", (128, 2), f32, kind="ExternalInput")
    o = [
        nc.dram_tensor(f"o{i}", (C, HS[i] + 4, W8[i]), f32, kind="ExternalOutput")
        for i in range(5)
    ]

    with TileContext(nc) as tc:
        with (
            tc.tile_pool(name="wp", bufs=1) as wpool,
            tc.tile_pool(name="xp", bufs=1) as xpool,
            tc.tile_pool(name="op", bufs=1) as opool,
            tc.tile_pool(name="ps", bufs=2, space="PSUM") as pspool,
        ):
            # conv weights + biases -> SBUF (scalar HWDGE ring, so the sync
            # ring can start streaming the bulk body copies immediately)
            wt = {}
            for name, t in [("A0", wA0), ("A1", wA1), ("B0", wB0), ("B1", wB1)]:
                tile = wpool.tile([128, 2, 5, 2, 128], f32, tag=f"w{name}")
                nc.scalar.dma_start(out=tile, in_=t[:, :, :, :, :])
                wt[name] = tile
            bt = {}
            for name, t in [("B0", bias0), ("B1", bias1)]:
                tile = wpool.tile([128, 2], f32, tag=f"bias{name}")
                nc.scalar.dma_start(out=tile, in_=t[:, :])
                bt[name] = tile

            def edge_A(eid, wtile, xin_rows, out_rows, Wz):
                # transposed conv, stride 2:
                #   z[2u] = K0@x[u] + K2@x[u+1] + K4@x[u+2]
                #   z[2u+1] = K1@x[u+1] + K3@x[u+2]     (x = haloed edge slice)
                Wx = Wz // 2 + 2
                xt = []
                for hi in range(2):
                    t = xpool.tile([128, 2, Wx], f32, tag=f"x{eid}_{hi}")
                    nc.scalar.dma_start(out=t, in_=xin_rows(hi))
                    xt.append(t)
                u = Wz // 2
                for ho in range(2):
                    pe = pspool.tile([128, 2, u], f32, tag="pe")
                    po = pspool.tile([128, 2, u], f32, tag="po")
                    seq_e = [(0, 0), (2, 1), (4, 2)]
                    for i, (m, sh) in enumerate(seq_e):
                        for hi in range(2):
                            nc.tensor.matmul(
                                pe,
                                wtile[:, hi, m, ho, :],
                                xt[hi][:, :, sh:sh + u],
                                start=(i == 0 and hi == 0),
                                stop=(i == 2 and hi == 1),
                            )
                    seq_o = [(1, 1), (3, 2)]
                    for i, (m, sh) in enumerate(seq_o):
                        for hi in range(2):
                            nc.tensor.matmul(
                                po,
                                wtile[:, hi, m, ho, :],
                                xt[hi][:, :, sh:sh + u],
                                start=(i == 0 and hi == 0),
                                stop=(i == 1 and hi == 1),
                            )
                    ot = opool.tile([128, 2, Wz], f32, tag=f"o{eid}_{ho}")
                    nc.vector.tensor_copy(out=ot[:, :, 0:Wz:2], in_=pe)
                    nc.vector.tensor_copy(out=ot[:, :, 1:Wz:2], in_=po)
                    nc.scalar.dma_start(out=out_rows(ho), in_=ot)

            def edge_B(eid, wtile, btile, e_dram, out_rows, Wk):
                # conv, stride 2: z[t] = sum_m Km@x[2t+m] + bias
                Wx = 2 * Wk + 3
                xt = []
                for hi in range(2):
                    t = xpool.tile([128, 2, Wx], f32, tag=f"x{eid}_{hi}")
                    nc.scalar.dma_start(
                        out=t, in_=e_dram[hi * 128:(hi + 1) * 128, :, :]
                    )
                    xt.append(t)
                for ho in range(2):
                    p = pspool.tile([128, 2, Wk], f32, tag="pb")
                    for m in range(5):
                        for hi in range(2):
                            nc.tensor.matmul(
                                p,
                                wtile[:, hi, m, ho, :],
                                xt[hi][:, :, m:m + 2 * Wk - 1:2],
                                start=(m == 0 and hi == 0),
                                stop=(m == 4 and hi == 1),
                            )
                    ot = opool.tile([128, 2, Wk], f32, tag=f"o{eid}_{ho}")
                    nc.vector.tensor_scalar_add(ot, p, btile[:, ho:ho + 1])
                    nc.scalar.dma_start(out=out_rows(ho), in_=ot)

            def rows(t, r0, r1):
                return lambda h: t[h * 128:(h + 1) * 128, r0:r1, :]

            edge_A("a1", wt["A0"], rows(b[1], 58, 60), rows(o[2], 0, 2), 180)
            edge_A("a2", wt["A0"], rows(b[3], 0, 2), rows(o[2], 182, 184), 180)
            edge_A("a3", wt["A1"], rows(b[0], 28, 30), rows(o[1], 0, 2), 90)
            edge_A("a4", wt["A1"], rows(b[4], 0, 2), rows(o[3], 62, 64), 90)
            edge_B("b5", wt["B0"], bt["B0"], e5, rows(o[1], 62, 64), 90)
            edge_B("b6", wt["B0"], bt["B0"], e6, rows(o[3], 0, 2), 90)
            edge_B("b7", wt["B1"], bt["B1"], e7, rows(o[0], 32, 34), 45)
            edge_B("b8", wt["B1"], bt["B1"], e8, rows(o[4], 0, 2), 45)

            # zero edges of the outermost strips (no coarser neighbor)
            zt = opool.tile([128, 2, 45], f32, tag="zeros")
            nc.vector.memset(zt, 0.0)
            for hi in range(2):
                nc.scalar.dma_start(
                    out=o[0][hi * 128:(hi + 1) * 128, 0:2, :], in_=zt
                )
                nc.scalar.dma_start(
                    out=o[4][hi * 128:(hi + 1) * 128, 32:34, :], in_=zt
                )

        # bulk body copies, DRAM -> DRAM (rows [2, H+2) of each output chunk),
        # on the sync HWDGE ring
        for i in (0, 1, 3, 4):
            nc.sync.dma_start(
                out=o[i][:, 2:HS[i] + 2, :], in_=b[i][:, :, 0:W8[i]]
            )
        for g in range(4):
            nc.sync.dma_start(
                out=o[2][64 * g:64 * (g + 1), 2:182, :],
                in_=b[2][64 * g:64 * (g + 1), :, 0:180],
            )

    nc.compile()
    return nc


def _slice_cols(arr, a, n):
    """arr[..., a:a+n] with circular wrap on the last axis; contiguous copy."""
    W = arr.shape[-1]
    a %= W
    if a + n <= W:
        return np.ascontiguousarray(arr[..., a:a + n])
    return np.concatenate([arr[..., a:], arr[..., :a + n - W]], axis=-1)


def _pack_A(wi):
    # lhsT for tap m of the transposed conv: lhsT_m[cin, cout] = wi[cin, cout, 0, 4-m]
    wiT = wi[:, :, 0, ::-1]
    return np.ascontiguousarray(
        wiT.reshape(2, 128, 2, 128, 5).transpose(1, 0, 4, 2, 3)
    )


def _pack_B(wo):
    # lhsT for tap m of the conv: lhsT_m[cin, cout] = wo[cout, cin, 0, m]
    wo_ = wo[:, :, 0, :]
    return np.ascontiguousarray(
        wo_.reshape(2, 128, 2, 128, 5).transpose(3, 2, 4, 0, 1)
    )


def kernel(s0, s1, s2, s3, s4, wi0, wi1, wo0, bo0, wo1, bo1):
    global _cached_nc
    from concourse.bass_utils import run_bass_kernel_spmd
    import kernel as _self

    if _cached_nc is None:
        _cached_nc = _build()
    nc = _cached_nc

    s = [np.asarray(x, dtype=np.float32)[0] for x in (s0, s1, s2, s3, s4)]
    wA0 = _pack_A(np.asarray(wi0, np.float32))
    wA1 = _pack_A(np.asarray(wi1, np.float32))
    wB0 = _pack_B(np.asarray(wo0, np.float32))
    wB1 = _pack_B(np.asarray(wo1, np.float32))
    bb0 = np.ascontiguousarray(np.asarray(bo0, np.float32).reshape(2, 128).T)
    bb1 = np.ascontiguousarray(np.asarray(bo1, np.float32).reshape(2, 128).T)

    in_maps = []
    for k in range(NCORES):
        m = {
            "wA0": wA0, "wA1": wA1, "wB0": wB0, "wB1": wB1,
            "bias0": bb0, "bias1": bb1,
        }
        for i in range(5):
            m[f"b{i}"] = _slice_cols(s[i], k * W8[i] - 2, W8[i] + 2)
        m["e5"] = _slice_cols(s[2][:, 0:2, :], 180 * k - 6, 183)
        m["e6"] = _slice_cols(s[2][:, 178:180, :], 180 * k - 6, 183)
        m["e7"] = _slice_cols(s[1][:, 0:2, :], 90 * k - 6, 93)
        m["e8"] = _slice_cols(s[3][:, 58:60, :], 90 * k - 6, 93)
        in_maps.append(m)

    res = run_bass_kernel_spmd(nc, in_maps, core_ids=list(range(NCORES)))
    _self.LAST_RESULT = res

    outs = []
    for i in range(5):
        w8 = W8[i]
        full = np.empty((1, C, HS[i] + 4, WS[i] + 4), np.float32)
        for k in range(NCORES):
            full[0, :, :, k * w8:(k + 1) * w8] = res.results[k][f"o{i}"]
        full[0, :, :, WS[i]:] = full[0, :, :, 0:4]
        outs.append(full)
    return tuple(outs)


LAST_RESULT = None


# revision 4
# speedup vs baseline: 2.4590x; 2.4590x over previous
"""Trainium2 Bass kernel for HarebrainedPad2d.

Five "earth" strips at different resolutions each get circular-W / zero-H
padding (PAD=2); the 2-row top/bottom pad edges are then overwritten with
small stride-2 convs (to_outer) / transposed convs (to_inner) of the
neighboring strips' edge rows.

Strategy: shard the longitude (W) axis across 8 NeuronCores. The host
slices each strip per core with a 2-column circular halo (so no device
collectives are needed), each core produces its exact W/8-column chunk of
all five padded outputs (bulk = DRAM->DRAM body copies; edges = TensorE
tap matmuls), and the host stitches the chunks back together. The padded
output's last 4 columns circularly duplicate its first 4, so the host
fills those by copying.
"""
import numpy as np

C = 256
HS = [30, 60, 180, 60, 30]
WS = [360, 720, 1440, 720, 360]
NCORES = 8
W8 = [w // 8 for w in WS]          # per-core output chunk widths [45,90,180,90,45]
BW = [w + 2 for w in W8]           # body input widths (2-col left halo + body)

_cached_nc = None


def _build():
    import concourse.bacc as bacc
    import concourse.mybir as mybir
    from concourse.tile import TileContext

    f32 = mybir.dt.float32
    nc = bacc.Bacc("TRN2")

    b = [
        nc.dram_tensor(f"b{i}", (C, HS[i], W8[i]), f32, kind="ExternalInput")
        for i in range(5)
    ]
    eA1 = nc.dram_tensor("eA1", (C, 2, 92), f32, kind="ExternalInput")
    eA2 = nc.dram_tensor("eA2", (C, 2, 92), f32, kind="ExternalInput")
    eA3 = nc.dram_tensor("eA3", (C, 2, 47), f32, kind="ExternalInput")
    eA4 = nc.dram_tensor("eA4", (C, 2, 47), f32, kind="ExternalInput")
    e5 = nc.dram_tensor("e5", (C, 2, 183), f32, kind="ExternalInput")
    e6 = nc.dram_tensor("e6", (C, 2, 183), f32, kind="ExternalInput")
    e7 = nc.dram_tensor("e7", (C, 2, 93), f32, kind="ExternalInput")
    e8 = nc.dram_tensor("e8", (C, 2, 93), f32, kind="ExternalInput")
    wA0 = nc.dram_tensor("wA0", (128, 2, 5, 2, 128), f32, kind="ExternalInput")
    wA1 = nc.dram_tensor("wA1", (128, 2, 5, 2, 128), f32, kind="ExternalInput")
    wB0 = nc.dram_tensor("wB0", (128, 2, 5, 2, 128), f32, kind="ExternalInput")
    wB1 = nc.dram_tensor("wB1", (128, 2, 5, 2, 128), f32, kind="ExternalInput")
    bias0 = nc.dram_tensor("bias0", (128, 2), f32, kind="ExternalInput")
    bias1 = nc.dram_tensor("bias1", (128, 2), f32, kind="ExternalInput")
    o = [
        nc.dram_tensor(f"o{i}", (C, HS[i] + 4, W8[i]), f32, kind="ExternalOutput")
        for i in range(5)
    ]

    with TileContext(nc) as tc:
        with (
            tc.tile_pool(name="wp", bufs=1) as wpool,
            tc.tile_pool(name="xp", bufs=1) as xpool,
            tc.tile_pool(name="op", bufs=1) as opool,
            tc.tile_pool(name="ps", bufs=2, space="PSUM") as pspool,
        ):
            # conv weights + biases -> SBUF (scalar HWDGE ring, so the sync
            # ring can start streaming the bulk body copies immediately)
            wt = {}
            for name, t in [("A0", wA0), ("A1", wA1), ("B0", wB0), ("B1", wB1)]:
                tile = wpool.tile([128, 2, 5, 2, 128], f32, tag=f"w{name}")
                nc.scalar.dma_start(out=tile, in_=t[:, :, :, :, :])
                wt[name] = tile
            bt = {}
            for name, t in [("B0", bias0), ("B1", bias1)]:
                tile = wpool.tile([128, 2], f32, tag=f"bias{name}")
                nc.scalar.dma_start(out=tile, in_=t[:, :])
                bt[name] = tile

            def edge_A(eid, wtile, xin_rows, out_rows, Wz):
                # transposed conv, stride 2:
                #   z[2u] = K0@x[u] + K2@x[u+1] + K4@x[u+2]
                #   z[2u+1] = K1@x[u+1] + K3@x[u+2]     (x = haloed edge slice)
                Wx = Wz // 2 + 2
                xt = []
                for hi in range(2):
                    t = xpool.tile([128, 2, Wx], f32, tag=f"x{eid}_{hi}")
                    nc.scalar.dma_start(out=t, in_=xin_rows(hi))
                    xt.append(t)
                u = Wz // 2
                for ho in range(2):
                    pe = pspool.tile([128, 2, u], f32, tag="pe")
                    po = pspool.tile([128, 2, u], f32, tag="po")
                    seq_e = [(0, 0), (2, 1), (4, 2)]
                    for i, (m, sh) in enumerate(seq_e):
                        for hi in range(2):
                            nc.tensor.matmul(
                                pe,
                                wtile[:, hi, m, ho, :],
                                xt[hi][:, :, sh:sh + u],
                                start=(i == 0 and hi == 0),
                                stop=(i == 2 and hi == 1),
                            )
                    seq_o = [(1, 1), (3, 2)]
                    for i, (m, sh) in enumerate(seq_o):
                        for hi in range(2):
                            nc.tensor.matmul(
                                po,
                                wtile[:, hi, m, ho, :],
                                xt[hi][:, :, sh:sh + u],
                                start=(i == 0 and hi == 0),
                                stop=(i == 1 and hi == 1),
                            )
                    ot = opool.tile([128, 2, Wz], f32, tag=f"o{eid}_{ho}")
                    nc.vector.tensor_copy(out=ot[:, :, 0:Wz:2], in_=pe)
                    nc.vector.tensor_copy(out=ot[:, :, 1:Wz:2], in_=po)
                    nc.scalar.dma_start(out=out_rows(ho), in_=ot)

            def edge_B(eid, wtile, btile, e_dram, out_rows, Wk):
                # conv, stride 2: z[t] = sum_m Km@x[2t+m] + bias
                Wx = 2 * Wk + 3
                xt = []
                for hi in range(2):
                    t = xpool.tile([128, 2, Wx], f32, tag=f"x{eid}_{hi}")
                    nc.scalar.dma_start(
                        out=t, in_=e_dram[hi * 128:(hi + 1) * 128, :, :]
                    )
                    xt.append(t)
                for ho in range(2):
                    p = pspool.tile([128, 2, Wk], f32, tag="pb")
                    for m in range(5):
                        for hi in range(2):
                            nc.tensor.matmul(
                                p,
                                wtile[:, hi, m, ho, :],
                                xt[hi][:, :, m:m + 2 * Wk - 1:2],
                                start=(m == 0 and hi == 0),
                                stop=(m == 4 and hi == 1),
                            )
                    ot = opool.tile([128, 2, Wk], f32, tag=f"o{eid}_{ho}")
                    nc.vector.tensor_scalar_add(ot, p, btile[:, ho:ho + 1])
                    nc.scalar.dma_start(out=out_rows(ho), in_=ot)

            def rows(t, r0, r1):
                return lambda h: t[h * 128:(h + 1) * 128, r0:r1, :]

            edge_A("a1", wt["A0"], rows(eA1, 0, 2), rows(o[2], 0, 2), 180)
            edge_A("a2", wt["A0"], rows(eA2, 0, 2), rows(o[2], 182, 184), 180)
            edge_A("a3", wt["A1"], rows(eA3, 0, 2), rows(o[1], 0, 2), 90)
            edge_A("a4", wt["A1"], rows(eA4, 0, 2), rows(o[3], 62, 64), 90)
            edge_B("b5", wt["B0"], bt["B0"], e5, rows(o[1], 62, 64), 90)
            edge_B("b6", wt["B0"], bt["B0"], e6, rows(o[3], 0, 2), 90)
            edge_B("b7", wt["B1"], bt["B1"], e7, rows(o[0], 32, 34), 45)
            edge_B("b8", wt["B1"], bt["B1"], e8, rows(o[4], 0, 2), 45)

            # zero edges of the outermost strips (no coarser neighbor)
            zt = opool.tile([128, 2, 45], f32, tag="zeros")
            nc.vector.memset(zt, 0.0)
            for hi in range(2):
                nc.scalar.dma_start(
                    out=o[0][hi * 128:(hi + 1) * 128, 0:2, :], in_=zt
                )
                nc.scalar.dma_start(
                    out=o[4][hi * 128:(hi + 1) * 128, 32:34, :], in_=zt
                )

        # bulk body copies, DRAM -> DRAM (rows [2, H+2) of each output chunk),
        # on the sync HWDGE ring
        for i in (0, 1, 3, 4):
            nc.sync.dma_start(out=o[i][:, 2:HS[i] + 2, :], in_=b[i][:, :, :])
        for g in range(4):
            nc.sync.dma_start(
                out=o[2][64 * g:64 * (g + 1), 2:182, :],
                in_=b[2][64 * g:64 * (g + 1), :, :],
            )

    nc.compile()
    return nc


def _slice_cols(arr, a, n):
    """arr[..., a:a+n] with circular wrap on the last axis; contiguous copy."""
    W = arr.shape[-1]
    a %= W
    if a + n <= W:
        return np.ascontiguousarray(arr[..., a:a + n])
    return np.concatenate([arr[..., a:], arr[..., :a + n - W]], axis=-1)


def _pack_A(wi):
    # lhsT for tap m of the transposed conv: lhsT_m[cin, cout] = wi[cin, cout, 0, 4-m]
    wiT = wi[:, :, 0, ::-1]
    return np.ascontiguousarray(
        wiT.reshape(2, 128, 2, 128, 5).transpose(1, 0, 4, 2, 3)
    )


def _pack_B(wo):
    # lhsT for tap m of the conv: lhsT_m[cin, cout] = wo[cout, cin, 0, m]
    wo_ = wo[:, :, 0, :]
    return np.ascontiguousarray(
        wo_.reshape(2, 128, 2, 128, 5).transpose(3, 2, 4, 0, 1)
    )


def kernel(s0, s1, s2, s3, s4, wi0, wi1, wo0, bo0, wo1, bo1):
    global _cached_nc
    from concourse.bass_utils import run_bass_kernel_spmd
    import kernel as _self

    if _cached_nc is None:
        _cached_nc = _build()
    nc = _cached_nc

    s = [np.asarray(x, dtype=np.float32)[0] for x in (s0, s1, s2, s3, s4)]
    wA0 = _pack_A(np.asarray(wi0, np.float32))
    wA1 = _pack_A(np.asarray(wi1, np.float32))
    wB0 = _pack_B(np.asarray(wo0, np.float32))
    wB1 = _pack_B(np.asarray(wo1, np.float32))
    bb0 = np.ascontiguousarray(np.asarray(bo0, np.float32).reshape(2, 128).T)
    bb1 = np.ascontiguousarray(np.asarray(bo1, np.float32).reshape(2, 128).T)

    in_maps = []
    for k in range(NCORES):
        m = {
            "wA0": wA0, "wA1": wA1, "wB0": wB0, "wB1": wB1,
            "bias0": bb0, "bias1": bb1,
        }
        for i in range(5):
            m[f"b{i}"] = _slice_cols(s[i], k * W8[i] - 2, W8[i])
        m["eA1"] = _slice_cols(s[1][:, 58:60, :], 90 * k - 2, 92)
        m["eA2"] = _slice_cols(s[3][:, 0:2, :], 90 * k - 2, 92)
        m["eA3"] = _slice_cols(s[0][:, 28:30, :], 45 * k - 2, 47)
        m["eA4"] = _slice_cols(s[4][:, 0:2, :], 45 * k - 2, 47)
        m["e5"] = _slice_cols(s[2][:, 0:2, :], 180 * k - 6, 183)
        m["e6"] = _slice_cols(s[2][:, 178:180, :], 180 * k - 6, 183)
        m["e7"] = _slice_cols(s[1][:, 0:2, :], 90 * k - 6, 93)
        m["e8"] = _slice_cols(s[3][:, 58:60, :], 90 * k - 6, 93)
        in_maps.append(m)

    res = run_bass_kernel_spmd(nc, in_maps, core_ids=list(range(NCORES)))
    _self.LAST_RESULT = res

    outs = []
    for i in range(5):
        w8 = W8[i]
        full = np.empty((1, C, HS[i] + 4, WS[i] + 4), np.float32)
        for k in range(NCORES):
            full[0, :, :, k * w8:(k + 1) * w8] = res.results[k][f"o{i}"]
        full[0, :, :, WS[i]:] = full[0, :, :, 0:4]
        outs.append(full)
    return tuple(outs)


LAST_RESULT = None
